# revision 1
# baseline (speedup 1.0000x reference)
"""Trainium2 Bass kernel: AttentionWithFeedForward (dense transformer block).

Sharding: 8 cores = (batch b = c//4) x (seq chunk of 1024 tokens = c%4).
Each core redundantly computes K/V over its full batch (no collectives),
Q/attention/FFN only for its own 1024-token chunk. The host rotates the
token axis per core so the own chunk is always columns 0:1024 (attention
is invariant to key order), keeping the device program identical across
cores.

Layout: all activations transposed [d_model, tok] ("ptile" layout
[128, d/128, tok]); host pre-transposes x/y and pre-casts weights to bf16.
Matmuls bf16 with fp32 PSUM accumulation. Softmax without max subtraction
(scores are tightly bounded at this problem's scale); denominators come
from a ones-column appended to V: even heads read AV output rows 0..64
with the denominator at row 64; odd heads use a 128-wide shifted view of
the packed [V|1] buffer so their output lands on partitions 64..127 with
the denominator at row 63 -- keeps VectorE lane alignment for the
normalize division.

SBUF is a two-sided stack allocator: frees must be LIFO per side, so big
tensors are placed left/right in nested lifetime order.
"""

from contextlib import ExitStack

import numpy as np
import ml_dtypes

import concourse.bass as bass
import concourse.tile as tile
from concourse import bacc, mybir
from concourse.bass_utils import run_bass_kernel_spmd

BF16 = mybir.dt.bfloat16
F32 = mybir.dt.float32
AF = mybir.ActivationFunctionType
OP = mybir.AluOpType

P = 128
D = 512          # d_embed
EJ = D // P      # 4 ptiles
DC = 768         # d_cross
CJ = DC // P     # 6
FF = 2048
FJ = FF // P     # 16
H = 8
DH = 64
S = 4096
ST = S // P      # 32 key tiles (full batch)
CH = 1024        # tokens per core
N2 = CH // 512   # 2 free-dim slices
B = 2
NCORES = 8
EPS = 1e-5
GELU_AF = AF.Gelu_apprx_tanh  # sim_test overrides with AF.Tanh (not in sim)

# bias_cols column layout; column j of a param holds param[128*j + p].
_BC = {}
_c = 0
for _nm, _n in [("qb", 4), ("kb", 4), ("vb", 4), ("saob", 4), ("caqb", 4),
                ("cakb", 4), ("cavb", 4), ("caob", 4), ("ffb1", 16),
                ("ffb2", 4), ("ln1g", 4), ("ln1b", 4), ("ln2g", 4),
                ("ln2b", 4), ("ln3g", 4), ("ln3b", 4)]:
    _BC[_nm] = (_c, _n)
    _c += _n
NBC = _c


def _pt(a):
    """[din, N] -> [128, din//128, N] ptile layout (partition-inner)."""
    din, n = a.shape
    return np.ascontiguousarray(a.reshape(din // P, P, n).transpose(1, 0, 2))


def _bcol(v):
    """[din] -> [128, din//128]."""
    return np.ascontiguousarray(v.reshape(-1, P).T)


def _bcast_ap(row_ap, nparts):
    """Broadcast a [1, N] DRAM AP across nparts partitions (step 0)."""
    return bass.AP(tensor=row_ap.tensor, offset=row_ap.offset,
                   ap=[[0, nparts]] + [list(d) for d in row_ap.ap[1:]])


def build(ctx, tc, dram):
    """Emit the full per-core program. Returns (names, out_name)."""
    nc = tc.nc
    names = {}

    def din(key, shape, dtype):
        t = dram.tile(shape, dtype, kind="ExternalInput", name=f"i_{key}")
        names[key] = t.name
        return t

    # ---- DRAM I/O ----
    xt_bf_d = din("xt_bf", [P, EJ, S], BF16)     # x[b].T rotated, bf16
    xt_f32_d = din("xt_f32", [P, EJ, CH], F32)   # own chunk (cols 0:CH), f32
    yt_d = din("yt", [P, CJ, 77], BF16)          # y[b].T
    w_qkv_d = din("w_qkv", [P, EJ, 3 * D], BF16)
    w_sao_d = din("w_sao", [P, EJ, D], BF16)
    w_caq_d = din("w_caq", [P, EJ, D], BF16)
    w_cak_d = din("w_cak", [P, CJ, D], BF16)
    w_cav_d = din("w_cav", [P, CJ, D], BF16)
    w_cao_d = din("w_cao", [P, EJ, D], BF16)
    w_ff1_d = din("w_ff1", [P, EJ, FF], BF16)
    w_ff2_d = din("w_ff2", [P, FJ, D], BF16)
    bias_d = din("bias", [P, NBC], F32)
    out_d = dram.tile([P, EJ, CH], F32, kind="ExternalOutput", name="o_out")
    out_name = out_d.name

    dma = nc.sync.dma_start

    def sb(key, shape, dtype, side):
        return tc.tile(shape, dtype, name=f"s_{key}", side=side)

    # ---- pools (never popped before build end) ----
    ps_a = ctx.enter_context(tc.tile_pool(name="ps_a", bufs=1, space="PSUM"))
    ps_o = ctx.enter_context(tc.tile_pool(name="ps_o", bufs=2, space="PSUM"))
    et_pool = ctx.enter_context(
        tc.tile_pool(name="et_pool", bufs=3, side="left"))
    rep_pool = ctx.enter_context(
        tc.tile_pool(name="rep_pool", bufs=2, side="left"))
    dsc_pool = ctx.enter_context(
        tc.tile_pool(name="dsc_pool", bufs=4, space="DRAM"))

    # ---- permanent small tiles (right-side bottom) ----
    bias_t, free_bias = sb("bias", [P, NBC], F32, "right")
    dma(out=bias_t[:, :], in_=bias_d[:, :])

    def bc(nm, j):
        c0, _n = _BC[nm]
        return bias_t[:, c0 + j:c0 + j + 1]

    ones_col, free_ones = sb("ones_col", [P, 1], BF16, "right")
    nc.vector.memset(ones_col[:, :], 1.0)
    eps_t, free_eps = sb("eps", [1, 1], F32, "right")
    nc.vector.memset(eps_t[:, :], EPS)
    yt, free_yt = sb("yt", [P, CJ, 77], BF16, "right")
    dma(out=yt[:, :, :], in_=yt_d[:, :, :])

    # ---- right stack: residual-stream tensors (freed at SA out-proj) ----
    xt_f32, free_xt_f32 = sb("xt_f32", [P, EJ, CH], F32, "right")
    dma(out=xt_f32[:, :, :], in_=xt_f32_d[:, :, :])
    ot, free_ot = sb("ot", [P, EJ, CH], BF16, "right")

    # ---- left stack: phase-1/SA tensors ----
    qt, free_qt = sb("qt", [P, EJ, CH], BF16, "left")
    kt, free_kt = sb("kt", [P, EJ, S], BF16, "left")
    v1, free_v1 = sb("v1", [P, ST, (H // 2) * 160], BF16, "left")
    xt_bf, free_xt_bf = sb("xt_bf", [P, EJ, S], BF16, "left")
    for e in range(EJ):
        dma(out=xt_bf[:, e, :], in_=xt_bf_d[:, e, :])
    w_qkv, free_w_qkv = sb("w_qkv", [P, EJ, 3 * D], BF16, "left")
    dma(out=w_qkv[:, :, :], in_=w_qkv_d[:, :, :])

    v1h = v1[:, :, :].rearrange("p t (pr c) -> p t pr c", c=160)
    nc.vector.memset(v1h[:, :, :, 64:65], 1.0)
    nc.vector.memset(v1h[:, :, :, 65:96], 0.0)

    # ---- phase 1: QKV projections (transposed layout) ----
    for j in range(EJ):
        for n in range(N2):
            ps = ps_o.tile([P, 2, 512], F32, tag="po", name="ps_q")
            for e in range(EJ):
                nc.tensor.matmul(
                    ps[:, 0, :], lhsT=w_qkv[:, e, P * j:P * (j + 1)],
                    rhs=xt_bf[:, e, 512 * n:512 * (n + 1)],
                    start=(e == 0), stop=(e == EJ - 1))
            nc.vector.tensor_scalar(out=qt[:, j, 512 * n:512 * (n + 1)],
                                    in0=ps[:, 0, :], scalar1=bc("qb", j),
                                    scalar2=None, op0=OP.add)
    for j in range(EJ):
        for n in range(S // 512):
            ps = ps_o.tile([P, 2, 512], F32, tag="po", name="ps_k")
            for e in range(EJ):
                nc.tensor.matmul(
                    ps[:, 0, :], lhsT=w_qkv[:, e, D + P * j:D + P * (j + 1)],
                    rhs=xt_bf[:, e, 512 * n:512 * (n + 1)],
                    start=(e == 0), stop=(e == EJ - 1))
            nc.vector.tensor_scalar(out=kt[:, j, 512 * n:512 * (n + 1)],
                                    in0=ps[:, 0, :], scalar1=bc("kb", j),
                                    scalar2=None, op0=OP.add)
    for t in range(ST):
        ps = ps_o.tile([P, 2, 512], F32, tag="po", name="ps_v")
        for e in range(EJ):
            nc.tensor.matmul(
                ps[:, 0, :], lhsT=xt_bf[:, e, P * t:P * (t + 1)],
                rhs=w_qkv[:, e, 2 * D:3 * D],
                start=(e == 0), stop=(e == EJ - 1))
        # V bias is applied after attention-normalize (per-partition there)
        psh = ps[:, 0, :].rearrange("p (pr two c) -> p pr two c", two=2, c=64)
        nc.vector.tensor_copy(out=v1h[:, t, :, 0:64], in_=psh[:, :, 0, :])
        nc.vector.tensor_copy(out=v1h[:, t, :, 96:160], in_=psh[:, :, 1, :])
    free_w_qkv()
    free_xt_bf()

    # ---- phase 2: self-attention, one head at a time ----
    def attn_pair(jp, kv_tiles, kp, kt_t, qt_t, v1_t, out_t, vb_nm):
        """One head pair (heads 2jp, 2jp+1): row-packed scores -> one wide
        exp -> AV for both heads -> per-head normalize into out_t."""
        o_a = ps_o.tile([65, 2, 512], F32, tag="po", name="o_even")
        o_b = ps_o.tile([P, 2, 512], F32, tag="po", name="o_odd")
        for kk in range(kv_tiles):
            sc = ps_a.tile([P, 4, 512], F32, tag="ps", name="sc")
            for n in range(N2):
                nc.tensor.matmul(
                    sc[0:kp, n, :],
                    lhsT=kt_t[0:DH, jp, P * kk:P * kk + kp],
                    rhs=qt_t[0:DH, jp, 512 * n:512 * (n + 1)],
                    start=True, stop=True)
                nc.tensor.matmul(
                    sc[0:kp, 2 + n, :],
                    lhsT=kt_t[DH:P, jp, P * kk:P * kk + kp],
                    rhs=qt_t[DH:P, jp, 512 * n:512 * (n + 1)],
                    start=True, stop=True)
            et = et_pool.tile([P, 4, 512], BF16, tag="et", name="et")
            nc.scalar.activation(et[0:kp, :, :], sc[0:kp, :, :], AF.Exp,
                                 scale=0.125)
            lhs_a = v1_t[0:kp, kk, 160 * jp:160 * jp + 65]
            lhs_b = v1_t[0:kp, kk, 160 * jp + 32:160 * jp + 160]
            for n in range(N2):
                nc.tensor.matmul(o_a[:, n, :], lhsT=lhs_a,
                                 rhs=et[0:kp, n, :],
                                 start=(kk == 0), stop=(kk == kv_tiles - 1))
                nc.tensor.matmul(o_b[:, n, :], lhsT=lhs_b,
                                 rhs=et[0:kp, 2 + n, :],
                                 start=(kk == 0), stop=(kk == kv_tiles - 1))
        for odd, o_ps, d_row in ((0, o_a, 64), (1, o_b, 32)):
            o_rows = o_ps[64:128, :, :] if odd else o_ps[0:64, :, :]
            # normalize: rep = 1/denom broadcast, out = O*rep
            rep = rep_pool.tile([P, 2, 512], F32, tag="rep", name="rep")
            nc.vector.tensor_copy(out=rep[d_row:d_row + 1, :, :],
                                  in_=o_ps[d_row:d_row + 1, :, :])
            # 1/den via exp(-ln(den)) on ScalarE (cheap vs DVE reciprocal)
            nc.scalar.activation(rep[d_row:d_row + 1, :, :],
                                 rep[d_row:d_row + 1, :, :], AF.Ln)
            nc.scalar.activation(rep[d_row:d_row + 1, :, :],
                                 rep[d_row:d_row + 1, :, :], AF.Exp,
                                 scale=-1.0)
            dsc = dsc_pool.tile([1, CH], F32, tag="dsc", name="dsc")
            dma(out=dsc[0:1, :],
                in_=rep[d_row:d_row + 1, :, :].rearrange("p a b -> p (a b)"))
            rrows = rep[64:128, :, :] if odd else rep[0:64, :, :]
            dma(out=rrows.rearrange("p a b -> p (a b)"),
                in_=_bcast_ap(dsc[0:1, :], 64))
            orng = slice(64, 128) if odd else slice(0, 64)
            nc.vector.tensor_tensor(
                out=out_t[orng, jp, :].rearrange("p (a b) -> p a b", b=512),
                in0=o_rows, in1=rrows, op=OP.mult)
        nc.vector.tensor_scalar(out=out_t[:, jp, :], in0=out_t[:, jp, :],
                                scalar1=bc(vb_nm, jp), scalar2=None,
                                op0=OP.add)

    for jp in range(H // 2):
        attn_pair(jp, ST, P, kt, qt, v1, ot, "vb")
    free_v1()
    free_kt()
    free_qt()

    def proj_resid(w_t, in_t, res_t, out_t, b_nm, kj):
        """out_t[:,j,:] (f32) = w_t.T @ in_t + bias + res_t  (kj ptiles)."""
        for j in range(EJ):
            for n in range(N2):
                ps = ps_o.tile([P, 2, 512], F32, tag="po", name="ps_pr")
                for e in range(kj):
                    nc.tensor.matmul(
                        ps[:, 0, :], lhsT=w_t[:, e, P * j:P * (j + 1)],
                        rhs=in_t[:, e, 512 * n:512 * (n + 1)],
                        start=(e == 0), stop=(e == kj - 1))
                sl = slice(512 * n, 512 * (n + 1))
                nc.vector.tensor_scalar(out=out_t[:, j, sl], in0=ps[:, 0, :],
                                        scalar1=bc(b_nm, j), scalar2=None,
                                        op0=OP.add)
                nc.vector.tensor_tensor(out=out_t[:, j, sl],
                                        in0=out_t[:, j, sl],
                                        in1=res_t[:, j, sl], op=OP.add)

    def layernorm(src_t, out_t, g_nm, b_nm, side):
        """LN over d (partitions x ptiles). src_t f32 [P,EJ,CH] (destroyed)."""
        xq, free_xq = sb(f"xq_{g_nm}", [P, EJ, CH], BF16, side)
        sq, free_sq = sb(f"sq_{g_nm}", [P, EJ, CH], BF16, side)
        st, free_st = sb(f"st_{g_nm}", [1, 3, CH], F32, side)
        nc.vector.tensor_copy(out=xq[:, :, :], in_=src_t[:, :, :])
        nc.vector.tensor_tensor(out=sq[:, :, :], in0=xq[:, :, :],
                                in1=xq[:, :, :], op=OP.mult)
        sums_m = ps_o.tile([1, 2, 512], F32, tag="po", name="sums_m")
        sums_s = ps_o.tile([1, 2, 512], F32, tag="po", name="sums_s")
        for dst, srct in ((sums_m, xq), (sums_s, sq)):
            for n in range(N2):
                for e in range(EJ):
                    nc.tensor.matmul(
                        dst[0:1, n, :], lhsT=ones_col[:, :],
                        rhs=srct[:, e, 512 * n:512 * (n + 1)],
                        start=(e == 0), stop=(e == EJ - 1))
        # st slots: 0 = mean, 1 = var -> rstd (in place), 2 = mean^2 tmp
        nc.vector.tensor_scalar(
            out=st[0:1, 0, :],
            in0=sums_m[0:1, :, :].rearrange("p a b -> p (a b)"),
            scalar1=1.0 / D, scalar2=None, op0=OP.mult)
        nc.vector.tensor_scalar(
            out=st[0:1, 1, :],
            in0=sums_s[0:1, :, :].rearrange("p a b -> p (a b)"),
            scalar1=1.0 / D, scalar2=None, op0=OP.mult)
        nc.vector.tensor_tensor(out=st[0:1, 2, :], in0=st[0:1, 0, :],
                                in1=st[0:1, 0, :], op=OP.mult)
        nc.vector.tensor_tensor(out=st[0:1, 1, :], in0=st[0:1, 1, :],
                                in1=st[0:1, 2, :], op=OP.subtract)
        # rstd = exp(-0.5 * ln(var + eps))  (stays in natural_log_exp set)
        nc.scalar.activation(st[0:1, 1, :], st[0:1, 1, :], AF.Ln,
                             bias=eps_t[0:1, :])
        nc.scalar.activation(st[0:1, 1, :], st[0:1, 1, :], AF.Exp, scale=-0.5)
        dsc = dsc_pool.tile([2, CH], F32, tag="dsc2", name="dsc2")
        dma(out=dsc[0:1, :], in_=st[0:1, 0, :])
        dma(out=dsc[1:2, :], in_=st[0:1, 1, :])
        rep_m = rep_pool.tile([P, 2, 512], F32, tag="rep", name="rep_m")
        rep_r = rep_pool.tile([P, 2, 512], F32, tag="rep", name="rep_r")
        dma(out=rep_m[:, :, :].rearrange("p a b -> p (a b)"),
            in_=_bcast_ap(dsc[0:1, :], P))
        dma(out=rep_r[:, :, :].rearrange("p a b -> p (a b)"),
            in_=_bcast_ap(dsc[1:2, :], P))
        for j in range(EJ):
            xv = src_t[:, j, :].rearrange("p (a b) -> p a b", b=512)
            nc.vector.tensor_tensor(out=xv, in0=xv, in1=rep_m[:, :, :],
                                    op=OP.subtract)
            nc.vector.tensor_tensor(out=xv, in0=xv, in1=rep_r[:, :, :],
                                    op=OP.mult)
            nc.vector.tensor_scalar(out=out_t[:, j, :], in0=src_t[:, j, :],
                                    scalar1=bc(g_nm, j), scalar2=bc(b_nm, j),
                                    op0=OP.mult, op1=OP.add)
        free_st()
        free_sq()
        free_xq()

    # ---- phase 3: SA out-proj + residual + LN1 ----
    x1, free_x1 = sb("x1", [P, EJ, CH], BF16, "left")
    xres, free_xres = sb("xres", [P, EJ, CH], F32, "left")
    w_sao, free_w_sao = sb("w_sao", [P, EJ, D], BF16, "left")
    dma(out=w_sao[:, :, :], in_=w_sao_d[:, :, :])
    proj_resid(w_sao, ot, xt_f32, xres, "saob", EJ)
    free_w_sao()
    free_ot()
    free_xt_f32()
    layernorm(xres, x1, "ln1g", "ln1b", "right")
    free_xres()

    # ---- phase 4: cross-attention ----
    x2, free_x2 = sb("x2", [P, EJ, CH], BF16, "right")
    x2res, free_x2res = sb("x2res", [P, EJ, CH], F32, "right")
    oct_, free_oct = sb("oct", [P, EJ, CH], BF16, "right")
    w_caq, free_w_caq = sb("w_caq", [P, EJ, D], BF16, "right")
    dma(out=w_caq[:, :, :], in_=w_caq_d[:, :, :])
    w_cak, free_w_cak = sb("w_cak", [P, CJ, D], BF16, "right")
    dma(out=w_cak[:, :, :], in_=w_cak_d[:, :, :])
    w_cav, free_w_cav = sb("w_cav", [P, CJ, D], BF16, "right")
    dma(out=w_cav[:, :, :], in_=w_cav_d[:, :, :])
    qc, free_qc = sb("qc", [P, EJ, CH], BF16, "right")
    kc, free_kc = sb("kc", [P, EJ, 77], BF16, "right")
    vc1, free_vc1 = sb("vc1", [77, 1, (H // 2) * 160], BF16, "right")

    for j in range(EJ):
        for n in range(N2):
            ps = ps_o.tile([P, 2, 512], F32, tag="po", name="ps_cq")
            for e in range(EJ):
                nc.tensor.matmul(
                    ps[:, 0, :], lhsT=w_caq[:, e, P * j:P * (j + 1)],
                    rhs=x1[:, e, 512 * n:512 * (n + 1)],
                    start=(e == 0), stop=(e == EJ - 1))
            nc.vector.tensor_scalar(out=qc[:, j, 512 * n:512 * (n + 1)],
                                    in0=ps[:, 0, :], scalar1=bc("caqb", j),
                                    scalar2=None, op0=OP.add)
    for j in range(EJ):
        ps = ps_o.tile([P, 2, 512], F32, tag="po", name="ps_ck")
        for e in range(CJ):
            nc.tensor.matmul(ps[:, 0, 0:77],
                             lhsT=w_cak[:, e, P * j:P * (j + 1)],
                             rhs=yt[:, e, :],
                             start=(e == 0), stop=(e == CJ - 1))
        nc.vector.tensor_scalar(out=kc[:, j, :], in0=ps[:, 0, 0:77],
                                scalar1=bc("cakb", j), scalar2=None,
                                op0=OP.add)
    vc1h = vc1[:, :, :].rearrange("p t (pr c) -> p t pr c", c=160)
    nc.vector.memset(vc1h[:, :, :, 64:65], 1.0)
    nc.vector.memset(vc1h[:, :, :, 65:96], 0.0)
    psv = ps_o.tile([P, 2, 512], F32, tag="po", name="ps_cv")
    for e in range(CJ):
        nc.tensor.matmul(psv[0:77, 0, :], lhsT=yt[:, e, :],
                         rhs=w_cav[:, e, :], start=(e == 0),
                         stop=(e == CJ - 1))
    psvh = psv[0:77, 0, :].rearrange("p (pr two c) -> p pr two c", two=2, c=64)
    nc.vector.tensor_copy(out=vc1h[:, 0, :, 0:64], in_=psvh[:, :, 0, :])
    nc.vector.tensor_copy(out=vc1h[:, 0, :, 96:160], in_=psvh[:, :, 1, :])

    for jp in range(H // 2):
        attn_pair(jp, 1, 77, kc, qc, vc1, oct_, "cavb")
    free_vc1()
    free_kc()
    free_qc()
    free_w_cav()
    free_w_cak()
    free_w_caq()

    w_cao, free_w_cao = sb("w_cao", [P, EJ, D], BF16, "right")
    dma(out=w_cao[:, :, :], in_=w_cao_d[:, :, :])
    proj_resid(w_cao, oct_, x1, x2res, "caob", EJ)
    free_w_cao()
    free_oct()
    free_x1()
    layernorm(x2res, x2, "ln2g", "ln2b", "right")
    free_x2res()

    # ---- phase 5: FFN ----
    x3res, free_x3res = sb("x3res", [P, EJ, CH], F32, "left")
    w_ff1, free_w_ff1 = sb("w_ff1", [P, EJ, FF], BF16, "left")
    dma(out=w_ff1[:, :, :], in_=w_ff1_d[:, :, :])
    w_ff2, free_w_ff2 = sb("w_ff2", [P, FJ, D], BF16, "left")
    dma(out=w_ff2[:, :, :], in_=w_ff2_d[:, :, :])
    hbf, free_hbf = sb("hbf", [P, FJ, CH], BF16, "left")
    for f in range(FJ):
        ps = ps_o.tile([P, 2, 512], F32, tag="po", name="ps_f1")
        for n in range(N2):
            for e in range(EJ):
                nc.tensor.matmul(
                    ps[:, n, :], lhsT=w_ff1[:, e, P * f:P * (f + 1)],
                    rhs=x2[:, e, 512 * n:512 * (n + 1)],
                    start=(e == 0), stop=(e == EJ - 1))
        nc.scalar.activation(
            hbf[:, f, :].rearrange("p (a b) -> p a b", b=512), ps[:, :, :],
            GELU_AF, bias=bc("ffb1", f))
    proj_resid(w_ff2, hbf, x2, x3res, "ffb2", FJ)
    free_hbf()
    free_w_ff2()
    free_w_ff1()
    free_x2()
    layernorm(x3res, x3res, "ln3g", "ln3b", "right")
    for j in range(EJ):
        dma(out=out_d[:, j, :], in_=x3res[:, j, :])
    free_x3res()
    free_yt()
    free_eps()
    free_ones()
    free_bias()

    return names, out_name


_CACHE = {}


def _compiled():
    if "nc" not in _CACHE:
        nc = bacc.Bacc("TRN2", target_bir_lowering=False, debug=False)
        with tile.TileContext(nc) as tc:
            with tc.tile_pool(name="dram_io", bufs=1, space="DRAM") as dram:
                with ExitStack() as ctx:
                    names, out_name = build(ctx, tc, dram)
        nc.compile()
        _CACHE["nc"] = (nc, names, out_name)
    return _CACHE["nc"]


def make_in_maps(inputs, names):
    """Host-side sharding: full inputs -> 8 per-core in_maps."""
    bf = ml_dtypes.bfloat16
    f32 = np.float32
    x = np.asarray(inputs["x"], f32)
    y = np.asarray(inputs["y"], f32)
    w = {k: np.asarray(v, f32) for k, v in inputs.items()}

    bias = np.zeros((P, NBC), f32)
    for nm, src in [("qb", w["sa_in_b"][0:D]), ("kb", w["sa_in_b"][D:2 * D]),
                    ("vb", w["sa_in_b"][2 * D:3 * D]), ("saob", w["sa_out_b"]),
                    ("caqb", w["ca_q_b"]), ("cakb", w["ca_k_b"]),
                    ("cavb", w["ca_v_b"]), ("caob", w["ca_out_b"]),
                    ("ffb1", w["ff_b1"]), ("ffb2", w["ff_b2"]),
                    ("ln1g", w["ln1_g"]), ("ln1b", w["ln1_b"]),
                    ("ln2g", w["ln2_g"]), ("ln2b", w["ln2_b"]),
                    ("ln3g", w["ln3_g"]), ("ln3b", w["ln3_b"])]:
        c0, n = _BC[nm]
        bias[:, c0:c0 + n] = _bcol(src)

    wt = {
        "w_qkv": _pt(w["sa_in_w"]).astype(bf),
        "w_sao": _pt(w["sa_out_w"]).astype(bf),
        "w_caq": _pt(w["ca_q_w"]).astype(bf),
        "w_cak": _pt(w["ca_k_w"]).astype(bf),
        "w_cav": _pt(w["ca_v_w"]).astype(bf),
        "w_cao": _pt(w["ca_out_w"]).astype(bf),
        "w_ff1": _pt(w["ff_w1"]).astype(bf),
        "w_ff2": _pt(w["ff_w2"]).astype(bf),
        "bias": bias,
    }

    in_maps = []
    for c in range(NCORES):
        b, ch = c // 4, c % 4
        q0 = CH * ch
        # rotate tokens so the own chunk sits at columns 0:CH
        xtb = np.roll(_pt(x[b].T), -q0, axis=2)    # [128, EJ, S] f32
        m = {names[k]: v for k, v in wt.items()}
        m[names["xt_bf"]] = xtb.astype(bf)
        m[names["xt_f32"]] = np.ascontiguousarray(xtb[:, :, 0:CH])
        m[names["yt"]] = _pt(y[b].T).astype(bf)
        in_maps.append(m)
    return in_maps


def assemble(results, out_name):
    out = np.zeros((B, S, D), np.float32)
    for c in range(NCORES):
        b, ch = c // 4, c % 4
        arr = np.asarray(results[c][out_name])     # [128, EJ, CH]
        out[b, CH * ch:CH * (ch + 1), :] = (
            arr.transpose(1, 0, 2).reshape(D, CH).T)
    return out


def run(inputs, **spmd_kwargs):
    nc, names, out_name = _compiled()
    in_maps = make_in_maps(inputs, names)
    res = run_bass_kernel_spmd(nc, in_maps, core_ids=list(range(NCORES)),
                               **spmd_kwargs)
    return assemble(res.results, out_name), res


def kernel(**inputs):
    out, _ = run(inputs)
    return out



# revision 2
# speedup vs baseline: 1.2267x; 1.2267x over previous
"""Trainium2 Bass kernel: AttentionWithFeedForward (dense transformer block).

Sharding: 8 cores = (batch b = c//4) x (seq chunk of 1024 tokens = c%4).
Each core redundantly computes K/V over its full batch (no collectives),
Q/attention/FFN only for its own 1024-token chunk. The host rotates the
token axis per core so the own chunk is always columns 0:1024 (attention
is invariant to key order), keeping the device program identical across
cores.

Layout: all activations transposed [d_model, tok] ("ptile" layout
[128, d/128, tok]); host pre-transposes x/y and pre-casts weights.

Perf structure vs the original version:
- Attention is pipelined per head: the scores PSUM tile is double-buffered
  (sc_pool bufs=2, 2 banks each) next to the AV accumulators (o_pool
  bufs=2, 2 banks each) = 8 PSUM banks exactly. The ACT engine then
  streams exps back-to-back while PE runs the next scores/AV matmuls,
  instead of ping-ponging (which also held PE at the mid p-state clock).
- The ACT engine executes ONLY Exp in the attention phases: softmax
  denominators are inverted with nc.vector.reciprocal on DVE and applied
  with a broadcast multiply, so no activation-table reloads interrupt the
  exp stream. LN rstd uses ACT Sqrt + DVE reciprocal.
- fp8 (e4m3) DoubleRow matmuls for QKV, AV, out-projs and the FFN:
  two 128-row k-tiles per instruction at 0.5 cycles/row = 4x bf16
  throughput. Weights are host-prescaled by 64 (keeps N(0,0.02) weights
  out of the fp8 subnormal range); the 1/64 descale folds into the bias
  tensor_scalar / gelu activation scale. V is packed at 8x into fp8; the
  softmax divide cancels the factor exactly (ones column is 8.0).
- Scores stay bf16 (contraction=64: head-even/odd matmuls already run
  concurrently in PE row quadrants). Residual stream stays f32/bf16.

Softmax without max subtraction (scores are tightly bounded here);
denominators come from a ones-column appended to V: even heads read AV
rows 0..64 with the denominator at row 64; odd heads use a 128-wide
shifted view of the packed [V|1] buffer so their output lands on
partitions 64..127 with the denominator at row 32 -- keeps VectorE lane
alignment for the normalize multiply.

SBUF is a two-sided stack allocator: frees must be LIFO per side.
"""

from contextlib import ExitStack

import numpy as np
import ml_dtypes

import concourse.bass as bass
import concourse.tile as tile
from concourse import bacc, mybir
from concourse.bass_utils import run_bass_kernel_spmd

BF16 = mybir.dt.bfloat16
F32 = mybir.dt.float32
F8 = mybir.dt.float8e4
AF = mybir.ActivationFunctionType
OP = mybir.AluOpType
DR = mybir.MatmulPerfMode.DoubleRow

P = 128
D = 512          # d_embed
EJ = D // P      # 4 ptiles
DC = 768         # d_cross
CJ = DC // P     # 6
FF = 2048
FJ = FF // P     # 16
H = 8
DH = 64
S = 4096
ST = S // P      # 32 key tiles (full batch)
CH = 1024        # tokens per core
N2 = CH // 512   # 2 free-dim slices
B = 2
NCORES = 8
EPS = 1e-5
WS = 64.0        # fp8 weight prescale (host side)
IWS = 1.0 / WS   # descale folded into bias pass
VS = 8.0 / WS    # V pack scale: stored V = 8*V_true (cancels in divide)
VONES = 8.0      # ones column matching the V scale
GELU_AF = AF.Gelu_apprx_tanh  # sim_test overrides with AF.Tanh (not in sim)

# bias_cols column layout; column j of a param holds param[128*j + p].
_BC = {}
_c = 0
for _nm, _n in [("qb", 4), ("kb", 4), ("vb", 4), ("saob", 4), ("caqb", 4),
                ("cakb", 4), ("cavb", 4), ("caob", 4), ("ffb1", 16),
                ("ffb2", 4), ("ln1g", 4), ("ln1b", 4), ("ln2g", 4),
                ("ln2b", 4), ("ln3g", 4), ("ln3b", 4)]:
    _BC[_nm] = (_c, _n)
    _c += _n
NBC = _c


def _pt(a):
    """[din, N] -> [128, din//128, N] ptile layout (partition-inner)."""
    din, n = a.shape
    return np.ascontiguousarray(a.reshape(din // P, P, n).transpose(1, 0, 2))


def _bcol(v):
    """[din] -> [128, din//128]."""
    return np.ascontiguousarray(v.reshape(-1, P).T)


def _bcast_ap(row_ap, nparts):
    """Broadcast a [1, N] DRAM AP across nparts partitions (step 0)."""
    return bass.AP(tensor=row_ap.tensor, offset=row_ap.offset,
                   ap=[[0, nparts]] + [list(d) for d in row_ap.ap[1:]])


def build(ctx, tc, dram):
    """Emit the full per-core program. Returns (names, out_name)."""
    nc = tc.nc
    names = {}

    def din(key, shape, dtype):
        t = dram.tile(shape, dtype, kind="ExternalInput", name=f"i_{key}")
        names[key] = t.name
        return t

    # ---- DRAM I/O ----
    xt8_d = din("xt8", [P, EJ, S], F8)           # x[b].T rotated, fp8
    xt_f32_d = din("xt_f32", [P, EJ, CH], F32)   # own chunk (cols 0:CH), f32
    yt_d = din("yt", [P, CJ, 77], BF16)          # y[b].T
    w_qkv_d = din("w_qkv8", [P, EJ, 3 * D], F8)  # fp8, x64
    w_sao_d = din("w_sao8", [P, EJ, D], F8)
    w_caq_d = din("w_caq8", [P, EJ, D], F8)
    w_cak_d = din("w_cak", [P, CJ, D], BF16)
    w_cav_d = din("w_cav", [P, CJ, D], BF16)
    w_cao_d = din("w_cao8", [P, EJ, D], F8)
    w_ff1_d = din("w_ff1", [P, EJ, FF], BF16)
    w_ff2_d = din("w_ff2", [P, FJ, D], BF16)
    bias_d = din("bias", [P, NBC], F32)
    out_d = dram.tile([P, EJ, CH], F32, kind="ExternalOutput", name="o_out")
    out_name = out_d.name

    dma = nc.sync.dma_start

    def sb(key, shape, dtype, side):
        return tc.tile(shape, dtype, name=f"s_{key}", side=side)

    # ---- pools ----
    # PSUM budget (8 banks): sc_pool 2x[128,2,512]f32 (4) + o_pool 2x (4).
    sc_pool = ctx.enter_context(
        tc.tile_pool(name="sc_pool", bufs=2, space="PSUM"))
    o_pool = ctx.enter_context(
        tc.tile_pool(name="o_pool", bufs=2, space="PSUM"))
    et_pool = ctx.enter_context(
        tc.tile_pool(name="et_pool", bufs=2, side="left"))
    etc_pool = ctx.enter_context(
        tc.tile_pool(name="etc_pool", bufs=2, side="left"))
    rep_pool = ctx.enter_context(
        tc.tile_pool(name="rep_pool", bufs=2, side="left"))
    row_pool = ctx.enter_context(
        tc.tile_pool(name="row_pool", bufs=2, side="left"))
    dsc_pool = ctx.enter_context(
        tc.tile_pool(name="dsc_pool", bufs=4, space="DRAM"))

    # ---- left stack: QKV-phase tensors ----
    qt, free_qt = sb("qt", [P, EJ, CH], BF16, "left")
    kt, free_kt = sb("kt", [P, EJ, S], BF16, "left")
    v1, free_v1 = sb("v1", [P, ST, (H // 2) * 160], F8, "left")
    xt8, free_xt8 = sb("xt8", [P, EJ, S], F8, "left")
    w_qkv, free_w_qkv = sb("w_qkv", [P, EJ, 3 * D], F8, "left")
    dma(out=w_qkv[:, :, :], in_=w_qkv_d[:, :, :])
    for e in range(EJ):
        dma(out=xt8[:, e, :], in_=xt8_d[:, e, :])

    # ---- permanent small tiles (right side) ----
    bias_t, free_bias = sb("bias", [P, NBC], F32, "right")
    dma(out=bias_t[:, :], in_=bias_d[:, :])

    def bc(nm, j):
        c0, _n = _BC[nm]
        return bias_t[:, c0 + j:c0 + j + 1]

    ones_col, free_ones = sb("ones_col", [P, 1], BF16, "right")
    nc.vector.memset(ones_col[:, :], 1.0)
    eps_t, free_eps = sb("eps", [1, 1], F32, "right")
    nc.vector.memset(eps_t[:, :], EPS)
    yt, free_yt = sb("yt", [P, CJ, 77], BF16, "right")
    dma(out=yt[:, :, :], in_=yt_d[:, :, :])
    xt_f32, free_xt_f32 = sb("xt_f32", [P, EJ, CH], F32, "right")
    dma(out=xt_f32[:, :, :], in_=xt_f32_d[:, :, :])
    ot, free_ot = sb("ot", [P, EJ, CH], F8, "right")
    w_sao, free_w_sao = sb("w_sao", [P, EJ, D], F8, "right")
    dma(out=w_sao[:, :, :], in_=w_sao_d[:, :, :])
    w_caq, free_w_caq = sb("w_caq", [P, EJ, D], F8, "right")
    dma(out=w_caq[:, :, :], in_=w_caq_d[:, :, :])
    w_cak, free_w_cak = sb("w_cak", [P, CJ, D], BF16, "right")
    dma(out=w_cak[:, :, :], in_=w_cak_d[:, :, :])
    w_cav, free_w_cav = sb("w_cav", [P, CJ, D], BF16, "right")
    dma(out=w_cav[:, :, :], in_=w_cav_d[:, :, :])
    w_cao, free_w_cao = sb("w_cao", [P, EJ, D], F8, "right")
    dma(out=w_cao[:, :, :], in_=w_cao_d[:, :, :])
    kc, free_kc = sb("kc", [P, EJ, 77], BF16, "right")
    vc1, free_vc1 = sb("vc1", [77, 1, (H // 2) * 160], BF16, "right")
    qc, free_qc = sb("qc", [P, EJ, CH], BF16, "right")
    oct_, free_oct = sb("oct", [P, EJ, CH], F8, "right")
    x1, free_x1 = sb("x1", [P, EJ, CH], BF16, "right")
    x1q, free_x1q = sb("x1q", [P, EJ, CH], F8, "right")
    x2, free_x2 = sb("x2", [P, EJ, CH], BF16, "right")


    v1h = v1[:, :, :].rearrange("p t (pr c) -> p t pr c", c=160)
    nc.vector.memset(v1h[:, :, :, 64:65], VONES)
    nc.vector.memset(v1h[:, :, :, 65:96], 0.0)

    # ---- phase 1: QKV projections (fp8 DoubleRow, transposed layout) ----
    for j in range(EJ):
        ps = sc_pool.tile([P, 2, 512], F32, tag="sc", name="ps_q")
        for n in range(N2):
            for ep in range(EJ // 2):
                nc.tensor.matmul(
                    ps[:, n, :],
                    lhsT=w_qkv[:, 2 * ep:2 * ep + 2, P * j:P * (j + 1)],
                    rhs=xt8[:, 2 * ep:2 * ep + 2, 512 * n:512 * (n + 1)],
                    start=(ep == 0), stop=(ep == EJ // 2 - 1), perf_mode=DR)
        nc.vector.tensor_scalar(
            out=qt[:, j, :], in0=ps[:, :, :].rearrange("p a b -> p (a b)"),
            scalar1=IWS, scalar2=bc("qb", j), op0=OP.mult, op1=OP.add)
    for j in range(EJ):
        for nn in range(S // CH):
            ps = sc_pool.tile([P, 2, 512], F32, tag="sc", name="ps_k")
            for n in range(N2):
                col = CH * nn + 512 * n
                for ep in range(EJ // 2):
                    nc.tensor.matmul(
                        ps[:, n, :],
                        lhsT=w_qkv[:, 2 * ep:2 * ep + 2,
                                   D + P * j:D + P * (j + 1)],
                        rhs=xt8[:, 2 * ep:2 * ep + 2, col:col + 512],
                        start=(ep == 0), stop=(ep == EJ // 2 - 1),
                        perf_mode=DR)
            nc.vector.tensor_scalar(
                out=kt[:, j, CH * nn:CH * (nn + 1)],
                in0=ps[:, :, :].rearrange("p a b -> p (a b)"),
                scalar1=IWS, scalar2=bc("kb", j), op0=OP.mult, op1=OP.add)
    # V: bias applied after attention-normalize; stored at 8x in fp8
    for tp in range(ST // 2):
        ps = sc_pool.tile([P, 2, 512], F32, tag="sc", name="ps_v")
        for tt in range(2):
            t = 2 * tp + tt
            for ep in range(EJ // 2):
                nc.tensor.matmul(
                    ps[:, tt, :],
                    lhsT=xt8[:, 2 * ep:2 * ep + 2, P * t:P * (t + 1)],
                    rhs=w_qkv[:, 2 * ep:2 * ep + 2, 2 * D:3 * D],
                    start=(ep == 0), stop=(ep == EJ // 2 - 1), perf_mode=DR)
        for tt in range(2):
            t = 2 * tp + tt
            psh = ps[:, tt, :].rearrange("p (pr two c) -> p pr two c",
                                         two=2, c=64)
            nc.vector.tensor_scalar(out=v1h[:, t, :, 0:64],
                                    in0=psh[:, :, 0, :], scalar1=VS,
                                    scalar2=None, op0=OP.mult)
            nc.vector.tensor_scalar(out=v1h[:, t, :, 96:160],
                                    in0=psh[:, :, 1, :], scalar1=VS,
                                    scalar2=None, op0=OP.mult)
    free_w_qkv()
    free_xt8()

    # ---- CA K/V projections (bf16, tiny; emitted early to overlap) ----
    for j in range(EJ):
        ps = sc_pool.tile([P, 2, 512], F32, tag="sc", name="ps_ck")
        for e in range(CJ):
            nc.tensor.matmul(ps[:, 0, 0:77],
                             lhsT=w_cak[:, e, P * j:P * (j + 1)],
                             rhs=yt[:, e, :],
                             start=(e == 0), stop=(e == CJ - 1))
        nc.vector.tensor_scalar(out=kc[:, j, :], in0=ps[:, 0, 0:77],
                                scalar1=bc("cakb", j), scalar2=None,
                                op0=OP.add)
    vc1h = vc1[:, :, :].rearrange("p t (pr c) -> p t pr c", c=160)
    nc.vector.memset(vc1h[:, :, :, 64:65], 1.0)
    nc.vector.memset(vc1h[:, :, :, 65:96], 0.0)
    psv = sc_pool.tile([P, 2, 512], F32, tag="sc", name="ps_cv")
    for e in range(CJ):
        nc.tensor.matmul(psv[0:77, 0, :], lhsT=yt[:, e, :],
                         rhs=w_cav[:, e, :], start=(e == 0),
                         stop=(e == CJ - 1))
    psvh = psv[0:77, 0, :].rearrange("p (pr two c) -> p pr two c", two=2, c=64)
    nc.vector.tensor_copy(out=vc1h[:, 0, :, 0:64], in_=psvh[:, :, 0, :])
    nc.vector.tensor_copy(out=vc1h[:, 0, :, 96:160], in_=psvh[:, :, 1, :])

    # ---- attention inner loop (shared SA/CA) ----
    def attn_head(h, kv_tiles, kp, kt_t, qt_t, v_t, out_t, doublerow):
        """One head: scores -> exp -> AV accumulate -> normalize (divide by
        the appended-ones denominator row via DVE reciprocal + broadcast).
        Even heads write out partitions 0:64 (denom at AV row 64), odd heads
        use the shifted 128-wide V view (data at 64:128, denom at row 32)."""
        jp, odd = divmod(h, 2)
        rows = slice(DH * odd, DH * odd + DH)
        o = o_pool.tile([P, 2, 512], F32, tag="o", name="o_att")
        if not odd:
            m_sl = slice(0, 65)
            v_off = 160 * jp
            v_w = 65
        else:
            m_sl = slice(0, 128)
            v_off = 160 * jp + 32
            v_w = 128
        if doublerow:
            for kkp in range(kv_tiles // 2):
                et = et_pool.tile([P, 2, 2, 512], F8, tag="et8", name="et")
                for t2 in range(2):
                    kk = 2 * kkp + t2
                    sc = sc_pool.tile([P, 2, 512], F32, tag="sc", name="sc")
                    for n in range(N2):
                        nc.tensor.matmul(
                            sc[0:kp, n, :],
                            lhsT=kt_t[rows, jp, P * kk:P * kk + kp],
                            rhs=qt_t[rows, jp, 512 * n:512 * (n + 1)],
                            start=True, stop=True)
                    nc.scalar.activation(et[0:kp, t2, :, :], sc[0:kp, :, :],
                                         AF.Exp, scale=0.125)
                lhs = v_t[0:kp, 2 * kkp:2 * kkp + 2, v_off:v_off + v_w]
                for n in range(N2):
                    nc.tensor.matmul(o[m_sl, n, :], lhsT=lhs,
                                     rhs=et[0:kp, :, n, :],
                                     start=(kkp == 0),
                                     stop=(kkp == kv_tiles // 2 - 1),
                                     perf_mode=DR)
        else:
            for kk in range(kv_tiles):
                et = etc_pool.tile([P, 2, 512], BF16, tag="etc", name="etc")
                sc = sc_pool.tile([P, 2, 512], F32, tag="sc", name="scc")
                for n in range(N2):
                    nc.tensor.matmul(
                        sc[0:kp, n, :],
                        lhsT=kt_t[rows, jp, P * kk:P * kk + kp],
                        rhs=qt_t[rows, jp, 512 * n:512 * (n + 1)],
                        start=True, stop=True)
                nc.scalar.activation(et[0:kp, :, :], sc[0:kp, :, :],
                                     AF.Exp, scale=0.125)
                lhs = v_t[0:kp, kk, v_off:v_off + v_w]
                for n in range(N2):
                    nc.tensor.matmul(o[m_sl, n, :], lhsT=lhs,
                                     rhs=et[0:kp, n, :],
                                     start=(kk == 0),
                                     stop=(kk == kv_tiles - 1))
        # normalize: recip of denom row on DVE, broadcast, multiply
        d_row = 64 if not odd else 32
        rrow = row_pool.tile([1, CH], F32, tag="rrow", name="rrow")
        nc.vector.reciprocal(
            out=rrow[0:1, :],
            in_=o[d_row:d_row + 1, :, :].rearrange("p a b -> p (a b)"))
        dr_t = dsc_pool.tile([1, CH], F32, tag="dsc", name="dsc")
        dma(out=dr_t[0:1, :], in_=rrow[0:1, :])
        rep = rep_pool.tile([P, 2, 512], F32, tag="rep", name="rep")
        orng = slice(0, 64) if not odd else slice(64, 128)
        rrows = rep[orng, :, :]
        dma(out=rrows.rearrange("p a b -> p (a b)"),
            in_=_bcast_ap(dr_t[0:1, :], 64))
        o_rows = o[0:64, :, :] if not odd else o[64:128, :, :]
        nc.vector.tensor_tensor(
            out=out_t[orng, jp, :].rearrange("p (a b) -> p a b", b=512),
            in0=o_rows, in1=rrows, op=OP.mult)

    # ---- phase 2: self-attention ----
    for h in range(H):
        attn_head(h, ST, P, kt, qt, v1, ot, True)
        if h % 2 == 1:
            jp = h // 2
            nc.vector.tensor_scalar(out=ot[:, jp, :], in0=ot[:, jp, :],
                                    scalar1=bc("vb", jp), scalar2=None,
                                    op0=OP.add)
    free_v1()
    free_kt()
    free_qt()

    def proj_resid(w_t, in_t, res_t, out_t, b_nm, kj, fp8=True):
        """out_t[:,j,:] (f32) = w_t.T @ in_t (descaled if fp8) + bias + res_t."""
        for j in range(EJ):
            ps = sc_pool.tile([P, 2, 512], F32, tag="sc", name="ps_pr")
            for n in range(N2):
                if fp8:
                    for ep in range(kj // 2):
                        nc.tensor.matmul(
                            ps[:, n, :],
                            lhsT=w_t[:, 2 * ep:2 * ep + 2, P * j:P * (j + 1)],
                            rhs=in_t[:, 2 * ep:2 * ep + 2,
                                     512 * n:512 * (n + 1)],
                            start=(ep == 0), stop=(ep == kj // 2 - 1),
                            perf_mode=DR)
                else:
                    for e in range(kj):
                        nc.tensor.matmul(
                            ps[:, n, :],
                            lhsT=w_t[:, e, P * j:P * (j + 1)],
                            rhs=in_t[:, e, 512 * n:512 * (n + 1)],
                            start=(e == 0), stop=(e == kj - 1))
            nc.vector.tensor_scalar(
                out=out_t[:, j, :],
                in0=ps[:, :, :].rearrange("p a b -> p (a b)"),
                scalar1=IWS if fp8 else 1.0, scalar2=bc(b_nm, j),
                op0=OP.mult, op1=OP.add)
            nc.vector.tensor_tensor(out=out_t[:, j, :], in0=out_t[:, j, :],
                                    in1=res_t[:, j, :], op=OP.add)

    def layernorm(src_t, out_t, out8_t, g_nm, b_nm, out_dma=None):
        """LN over d. src_t f32 [P,EJ,CH] (destroyed). out_t bf16 or f32;
        out8_t optional fp8 copy. rstd = 1/sqrt(var+eps) via ACT Sqrt +
        DVE reciprocal (no Ln/Exp -> no act-table churn)."""
        xq, free_xq = sb(f"xq_{g_nm}", [P, EJ, CH], BF16, "left")
        sq, free_sq = sb(f"sq_{g_nm}", [P, EJ, CH], BF16, "left")
        st, free_st = sb(f"st_{g_nm}", [1, 3, CH], F32, "left")
        nc.vector.tensor_copy(out=xq[:, :, :], in_=src_t[:, :, :])
        nc.vector.tensor_tensor(out=sq[:, :, :], in0=xq[:, :, :],
                                in1=xq[:, :, :], op=OP.mult)
        sums_m = o_pool.tile([1, 2, 512], F32, tag="o", name="sums_m")
        sums_s = o_pool.tile([1, 2, 512], F32, tag="o", name="sums_s")
        for dst, srct in ((sums_m, xq), (sums_s, sq)):
            for n in range(N2):
                for e in range(EJ):
                    nc.tensor.matmul(
                        dst[0:1, n, :], lhsT=ones_col[:, :],
                        rhs=srct[:, e, 512 * n:512 * (n + 1)],
                        start=(e == 0), stop=(e == EJ - 1))
        # st rows: 0 = mean, 1 = var -> std, 2 = mean^2 tmp -> rstd
        nc.vector.tensor_scalar(
            out=st[0:1, 0, :],
            in0=sums_m[0:1, :, :].rearrange("p a b -> p (a b)"),
            scalar1=1.0 / D, scalar2=None, op0=OP.mult)
        nc.vector.tensor_scalar(
            out=st[0:1, 1, :],
            in0=sums_s[0:1, :, :].rearrange("p a b -> p (a b)"),
            scalar1=1.0 / D, scalar2=None, op0=OP.mult)
        nc.vector.tensor_tensor(out=st[0:1, 2, :], in0=st[0:1, 0, :],
                                in1=st[0:1, 0, :], op=OP.mult)
        nc.vector.tensor_tensor(out=st[0:1, 1, :], in0=st[0:1, 1, :],
                                in1=st[0:1, 2, :], op=OP.subtract)
        nc.scalar.activation(st[0:1, 1, :], st[0:1, 1, :], AF.Sqrt,
                             bias=eps_t[0:1, :])
        nc.vector.reciprocal(out=st[0:1, 2, :], in_=st[0:1, 1, :])
        dsc = dsc_pool.tile([2, CH], F32, tag="dsc2", name="dsc2")
        dma(out=dsc[0:1, :], in_=st[0:1, 0, :])
        dma(out=dsc[1:2, :], in_=st[0:1, 2, :])
        rep_m = rep_pool.tile([P, 2, 512], F32, tag="rep", name="rep_m")
        rep_r = rep_pool.tile([P, 2, 512], F32, tag="rep", name="rep_r")
        dma(out=rep_m[:, :, :].rearrange("p a b -> p (a b)"),
            in_=_bcast_ap(dsc[0:1, :], P))
        dma(out=rep_r[:, :, :].rearrange("p a b -> p (a b)"),
            in_=_bcast_ap(dsc[1:2, :], P))
        for j in range(EJ):
            xv = src_t[:, j, :].rearrange("p (a b) -> p a b", b=512)
            nc.vector.tensor_tensor(out=xv, in0=xv, in1=rep_m[:, :, :],
                                    op=OP.subtract)
            nc.vector.tensor_tensor(out=xv, in0=xv, in1=rep_r[:, :, :],
                                    op=OP.mult)
            nc.vector.tensor_scalar(out=out_t[:, j, :], in0=src_t[:, j, :],
                                    scalar1=bc(g_nm, j), scalar2=bc(b_nm, j),
                                    op0=OP.mult, op1=OP.add)
            if out8_t is not None:
                nc.vector.tensor_copy(out=out8_t[:, j, :], in_=out_t[:, j, :])
            if out_dma is not None:
                dma(out=out_dma[:, j, :], in_=out_t[:, j, :])
        free_st()
        free_sq()
        free_xq()

    # ---- phase 3: SA out-proj + residual + LN1 ----
    xres, free_xres = sb("xres", [P, EJ, CH], F32, "left")
    proj_resid(w_sao, ot, xt_f32, xres, "saob", EJ)
    layernorm(xres, x1, x1q, "ln1g", "ln1b")
    free_xres()

    # ---- phase 4: cross-attention ----
    for j in range(EJ):
        ps = sc_pool.tile([P, 2, 512], F32, tag="sc", name="ps_cq")
        for n in range(N2):
            for ep in range(EJ // 2):
                nc.tensor.matmul(
                    ps[:, n, :],
                    lhsT=w_caq[:, 2 * ep:2 * ep + 2, P * j:P * (j + 1)],
                    rhs=x1q[:, 2 * ep:2 * ep + 2, 512 * n:512 * (n + 1)],
                    start=(ep == 0), stop=(ep == EJ // 2 - 1), perf_mode=DR)
        nc.vector.tensor_scalar(
            out=qc[:, j, :], in0=ps[:, :, :].rearrange("p a b -> p (a b)"),
            scalar1=IWS, scalar2=bc("caqb", j), op0=OP.mult, op1=OP.add)

    for h in range(H):
        attn_head(h, 1, 77, kc, qc, vc1, oct_, False)
        if h % 2 == 1:
            jp = h // 2
            nc.vector.tensor_scalar(out=oct_[:, jp, :], in0=oct_[:, jp, :],
                                    scalar1=bc("cavb", jp), scalar2=None,
                                    op0=OP.add)

    # FFN weights (bf16 for accuracy): start the DMA while CA executes
    w_ff1, free_w_ff1 = sb("w_ff1", [P, EJ, FF], BF16, "left")
    w_ff2, free_w_ff2 = sb("w_ff2", [P, FJ, D], BF16, "left")
    dma(out=w_ff1[:, :, :], in_=w_ff1_d[:, :, :])
    dma(out=w_ff2[:, :, :], in_=w_ff2_d[:, :, :])

    x2res, free_x2res = sb("x2res", [P, EJ, CH], F32, "left")
    proj_resid(w_cao, oct_, x1, x2res, "caob", EJ)
    layernorm(x2res, x2, None, "ln2g", "ln2b")
    free_x2res()

    # ---- phase 5: FFN (bf16) ----
    hbf, free_hbf = sb("hbf", [P, FJ, CH], BF16, "left")
    for f in range(FJ):
        ps = sc_pool.tile([P, 2, 512], F32, tag="sc", name="ps_f1")
        for n in range(N2):
            for e in range(EJ):
                nc.tensor.matmul(
                    ps[:, n, :],
                    lhsT=w_ff1[:, e, P * f:P * (f + 1)],
                    rhs=x2[:, e, 512 * n:512 * (n + 1)],
                    start=(e == 0), stop=(e == EJ - 1))
        nc.scalar.activation(
            hbf[:, f, :].rearrange("p (a b) -> p a b", b=512), ps[:, :, :],
            GELU_AF, bias=bc("ffb1", f))
    x3res, free_x3res = sb("x3res", [P, EJ, CH], F32, "left")
    proj_resid(w_ff2, hbf, x2, x3res, "ffb2", FJ, fp8=False)
    layernorm(x3res, x3res, None, "ln3g", "ln3b", out_dma=out_d)
    free_x3res()
    free_hbf()
    free_w_ff2()
    free_w_ff1()
    free_x2()
    free_x1q()
    free_x1()
    free_oct()
    free_qc()
    free_vc1()
    free_kc()
    free_w_cao()
    free_w_cav()
    free_w_cak()
    free_w_caq()
    free_w_sao()
    free_ot()
    free_xt_f32()
    free_yt()
    free_eps()
    free_ones()
    free_bias()

    return names, out_name


_CACHE = {}


def _compiled():
    if "nc" not in _CACHE:
        nc = bacc.Bacc("TRN2", target_bir_lowering=False, debug=False)
        with tile.TileContext(nc) as tc:
            with tc.tile_pool(name="dram_io", bufs=1, space="DRAM") as dram:
                with ExitStack() as ctx:
                    names, out_name = build(ctx, tc, dram)
        nc.compile()
        _CACHE["nc"] = (nc, names, out_name)
    return _CACHE["nc"]


def make_in_maps(inputs, names):
    """Host-side sharding: full inputs -> 8 per-core in_maps."""
    bf = ml_dtypes.bfloat16
    f8 = ml_dtypes.float8_e4m3
    f32 = np.float32
    x = np.asarray(inputs["x"], f32)
    y = np.asarray(inputs["y"], f32)
    w = {k: np.asarray(v, f32) for k, v in inputs.items()}

    bias = np.zeros((P, NBC), f32)
    for nm, src in [("qb", w["sa_in_b"][0:D]), ("kb", w["sa_in_b"][D:2 * D]),
                    ("vb", w["sa_in_b"][2 * D:3 * D]), ("saob", w["sa_out_b"]),
                    ("caqb", w["ca_q_b"]), ("cakb", w["ca_k_b"]),
                    ("cavb", w["ca_v_b"]), ("caob", w["ca_out_b"]),
                    ("ffb1", w["ff_b1"]), ("ffb2", w["ff_b2"]),
                    ("ln1g", w["ln1_g"]), ("ln1b", w["ln1_b"]),
                    ("ln2g", w["ln2_g"]), ("ln2b", w["ln2_b"]),
                    ("ln3g", w["ln3_g"]), ("ln3b", w["ln3_b"])]:
        c0, n = _BC[nm]
        bias[:, c0:c0 + n] = _bcol(src)

    wt = {
        "w_qkv8": (_pt(w["sa_in_w"]) * WS).astype(f8),
        "w_sao8": (_pt(w["sa_out_w"]) * WS).astype(f8),
        "w_caq8": (_pt(w["ca_q_w"]) * WS).astype(f8),
        "w_cak": _pt(w["ca_k_w"]).astype(bf),
        "w_cav": _pt(w["ca_v_w"]).astype(bf),
        "w_cao8": (_pt(w["ca_out_w"]) * WS).astype(f8),
        "w_ff1": _pt(w["ff_w1"]).astype(bf),
        "w_ff2": _pt(w["ff_w2"]).astype(bf),
        "bias": bias,
    }

    in_maps = []
    for c in range(NCORES):
        b, ch = c // 4, c % 4
        q0 = CH * ch
        # rotate tokens so the own chunk sits at columns 0:CH
        xtb = np.roll(_pt(x[b].T), -q0, axis=2)    # [128, EJ, S] f32
        m = {names[k]: v for k, v in wt.items()}
        m[names["xt8"]] = xtb.astype(f8)
        m[names["xt_f32"]] = np.ascontiguousarray(xtb[:, :, 0:CH])
        m[names["yt"]] = _pt(y[b].T).astype(bf)
        in_maps.append(m)
    return in_maps


def assemble(results, out_name):
    out = np.zeros((B, S, D), np.float32)
    for c in range(NCORES):
        b, ch = c // 4, c % 4
        arr = np.asarray(results[c][out_name])     # [128, EJ, CH]
        out[b, CH * ch:CH * (ch + 1), :] = (
            arr.transpose(1, 0, 2).reshape(D, CH).T)
    return out


def run(inputs, **spmd_kwargs):
    nc, names, out_name = _compiled()
    in_maps = make_in_maps(inputs, names)
    res = run_bass_kernel_spmd(nc, in_maps, core_ids=list(range(NCORES)),
                               **spmd_kwargs)
    return assemble(res.results, out_name), res


def kernel(**inputs):
    out, _ = run(inputs)
    return out
